# revision 14
# baseline (speedup 1.0000x reference)
"""Bass/Tile Trainium2 kernel for CrossPositionalAttention (v3: pipelined f32r).

Reference math (per batch element b):
    M = F @ W_M; N = F @ W_N; V = F @ W_V          # [T, C] each, T=2048, C=64
    S = softmax(M @ N^T, axis=-1)                  # [T, T]
    out = S @ V + F

Sharding: data-parallel over batch. B=8 == n_cores=8, so core i computes
batch element i end-to-end (no collectives); kernel() shards/gathers on host.

Design notes (v4):
  * Moving-operand bandwidth is ~2 B/lane/PE-cycle: bf16 rhs streams 1
    col/cycle at the full 2.4GHz clock, while f32r rhs is capped at the
    ~1.2GHz-equivalent rate (measured v3). So the inner loop uses bf16:
    scores = 3 bf16 passes on hi/lo splits (hi = bf16(x), lo = bf16(x-hi),
    ~17 bit combined: Nh.T@Mh + Nh.T@Ml + Nl.T@Mh), and PV consumes bf16
    expS with bf16 V. Preamble matmuls (projections, V natural, F^T
    transposes) stay f32r (f32r operands must be PRODUCED as f32r per the
    BIR verifier, hence bitcast dram APs / f32r-writing copies).
  * Permuted row order for fast DMA: F_sb[p, x, c] = F[16p + x, c] -- each
    partition loads 4KB contiguous. The permutation is applied consistently
    to M/N/V/scores/out and softmax is permutation-invariant over k, so
    writing out through the same view restores order.
  * ACT (scalar) does exp exclusively: [128,1024] psum->sbuf f32r per
    (qc,kp), ~1.04us saturated => ~33us floor; everything else is kept off
    ACT and the whole schedule aims to keep ACT saturated.
  * Software pipelining: PV(kp) is emitted after scores(kp+1) so the PE
    never waits on the exp it just requested; epilogue transposes of qc are
    jammed between the first kps of qc+1. The PE stays busy, which also
    holds the DVFS clock up (cold PE runs at 0.65GHz, warm at 2.4GHz).
  * Warmup: a few dummy ident transposes while the F DMA lands, so the
    PE clock ramps before the real preamble; a dummy exp on the scalar
    queue front-loads the ACT table load.
  * Preamble (F^T transposes -> M/N/V projections -> V natural transposes,
    by 4-block group) is interleaved with qc0's kp stream: group g is
    emitted before kps 2g, 2g+1 which consume it. psum->sbuf copies split
    between DVE and Pool so neither starves ACT.
  * DMA posting costs ~650ns per descriptor on an engine queue, so posts
    are spread: sync takes F chunks 0/2 + all output blocks, scalar (idle
    pre-exp) takes the weights and F chunks 1/3.
"""

import numpy as np

import concourse.bacc as bacc
import concourse.bass as bass
import concourse.tile as tile
from concourse import mybir
from concourse.bass_utils import run_bass_kernel_spmd
from concourse.masks import make_identity

B, T, C = 8, 2048, 64
P = 128
NBLK = T // P          # 16 blocks of 128 rows (permuted order)
QCHUNK = 512           # moving-operand free dim per matmul
NQC = T // QCHUNK      # 4 q-chunks
NG = 4                 # preamble block group size (4 blocks = 512 cols)
NKP = NBLK // 2        # 8 kp pairs per q-chunk
F32 = mybir.dt.float32
F32R = mybir.dt.float32r
BF16 = mybir.dt.bfloat16
EXP_BIAS = -40.0       # constant softmax shift (cancels in the normalization)
VPAD = 66              # V tile free dim: 64 V cols + ones col + pad (f32r: even)
NWARM = 3              # dummy transposes to ramp the PE clock


def build_nc() -> bass.Bass:
    nc = bacc.Bacc()
    F_h = nc.declare_dram_parameter("F", [T, C], F32, isOutput=False)
    Wm_h = nc.declare_dram_parameter("W_M", [C, C], F32, isOutput=False)
    Wn_h = nc.declare_dram_parameter("W_N", [C, C], F32, isOutput=False)
    Wv_h = nc.declare_dram_parameter("W_V", [C, C], F32, isOutput=False)
    out_h = nc.declare_dram_parameter("out", [T, C], F32, isOutput=True)

    # contiguous view: partition p holds rows 16p..16p+15 (4KB/partition)
    F_view = F_h[:, :].rearrange("(p x) c -> p x c", p=P)
    out_view = out_h[:, :].rearrange("(p x) c -> p x c", p=P)

    def r(ap):
        return ap.bitcast(F32R)

    def f(ap):
        return ap.bitcast(F32)

    with tile.TileContext(nc) as tc:
        with (
            tc.tile_pool(name="const", bufs=1) as const_pool,
            tc.tile_pool(name="persist", bufs=1) as persist,
            tc.tile_pool(name="sc_ps", bufs=2, space="PSUM") as sc_pool,
            tc.tile_pool(name="pv_ps", bufs=2, space="PSUM") as pv_pool,
            tc.tile_pool(name="misc_ps", bufs=2, space="PSUM") as misc,
            tc.tile_pool(name="work", bufs=3) as work,
            tc.tile_pool(name="ep", bufs=4) as ep,
        ):
            ident = const_pool.tile([P, P], F32, tag="ident")
            make_identity(nc, ident)
            ident_r = const_pool.tile([P, P], F32R, tag="identr")
            nc.vector.tensor_copy(ident_r, ident)

            exp_bias = const_pool.tile([P, 1], F32, tag="expbias")
            nc.vector.memset(exp_bias, EXP_BIAS)

            # F chunks alternate sync/scalar so posting parallelizes;
            # scalar (idle until first exp) also takes the weights
            F_sb = persist.tile([P, NBLK, C], F32R, tag="fsb")
            for g in range(NQC):
                eng = nc.sync if g % 2 == 0 else nc.scalar
                eng.dma_start(
                    out=F_sb[:, NG * g : NG * (g + 1), :],
                    in_=r(F_view[:, NG * g : NG * (g + 1), :]),
                )
            Wm2 = const_pool.tile([C, P], F32R, tag="wm2")
            Wn2 = const_pool.tile([C, P], F32R, tag="wn2")
            Wv_sb = const_pool.tile([C, C], F32R, tag="wv")
            nc.scalar.dma_start(out=Wm2[:, 0:C], in_=r(Wm_h[:, :]))
            nc.scalar.dma_start(out=Wm2[:, C:P], in_=r(Wm_h[:, :]))
            nc.scalar.dma_start(out=Wn2[:, 0:C], in_=r(Wn_h[:, :]))
            nc.scalar.dma_start(out=Wn2[:, C:P], in_=r(Wn_h[:, :]))
            nc.scalar.dma_start(out=Wv_sb[:, :], in_=r(Wv_h[:, :]))

            F_T = persist.tile([C, T], F32R, tag="ft")
            MTh = persist.tile([P, T], BF16, tag="mth")
            MTl = persist.tile([P, T], BF16, tag="mtl")
            NTh = persist.tile([P, T], BF16, tag="nth")
            NTl = persist.tile([P, T], BF16, tag="ntl")
            V_sb = persist.tile([P, NBLK, VPAD], BF16, tag="vsb")
            o_sb = persist.tile([P, NBLK, C], F32, tag="osb")

            # ones cols (64: softmax denominator via PV matmul, 65: pad for
            # f32r alignment), one strided copy from an fp32 ones tile
            ones32 = const_pool.tile([P, 2 * NBLK], F32, tag="ones32")
            nc.vector.memset(ones32, 1.0)
            nc.vector.tensor_copy(
                V_sb[:, :, C:VPAD],
                ones32.rearrange("p (n t) -> p n t", n=NBLK),
            )

            # dummy exp front-loads the ACT function-table load
            dumex = const_pool.tile([P, 2], BF16, tag="dumex")
            nc.scalar.activation(
                dumex, ones32[:, 0:2], mybir.ActivationFunctionType.Exp
            )

            # dummy transposes ramp the PE clock while the F DMA lands
            # (all misc psum tiles share one tag => one 2-slot ring, 2 banks)
            for w in range(NWARM):
                mx = misc.tile([P, QCHUNK], F32R, tag="mx", name=f"warm{w}")
                nc.tensor.transpose(mx[:, 0:P], ident_r, ident_r)

            def preamble_group(g):
                """F^T, M^T/N^T/VT projections and V natural for blocks
                4g..4g+3. psum->sbuf copies alternate DVE / Pool."""
                gsl = slice(g * QCHUNK, (g + 1) * QCHUNK)
                mx = misc.tile([P, QCHUNK], F32R, tag="mx", name=f"ftr{g}")
                ftr = mx[0:C, :]
                for j in range(NG):
                    blk = NG * g + j
                    nc.tensor.transpose(
                        ftr[:, j * P : (j + 1) * P], F_sb[:, blk, :], ident_r
                    )
                nc.vector.tensor_copy(F_T[:, gsl], ftr)

                for W2, hi, lo, pnm in (
                    (Wm2, MTh, MTl, "pm"),
                    (Wn2, NTh, NTl, "pn"),
                ):
                    ppx = misc.tile(
                        [P, QCHUNK], F32R, tag="mx", name=f"{pnm}{g}"
                    )
                    pp = f(ppx)
                    nc.tensor.matmul(
                        pp, lhsT=W2, rhs=F_T[:, gsl], start=True, stop=True
                    )
                    nc.vector.tensor_copy(hi[:, gsl], pp)
                    # lo = bf16(psum - hi) in one DVE op (bf16 output)
                    nc.vector.tensor_tensor(
                        out=lo[:, gsl],
                        in0=pp,
                        in1=hi[:, gsl],
                        op=mybir.AluOpType.subtract,
                    )

                # V natural directly: lhsT = F^T block (stationary), rhs =
                # W_V streamed (64 cols, f32r small-ap penalty is fine)
                vnx = misc.tile([P, QCHUNK], F32R, tag="mx", name=f"vn{g}")
                vn = f(vnx)[:, 0 : NG * C]
                for j in range(NG):
                    blk = NG * g + j
                    nc.tensor.matmul(
                        vn[:, j * C : (j + 1) * C],
                        lhsT=F_T[:, blk * P : (blk + 1) * P],
                        rhs=Wv_sb,
                        start=True,
                        stop=True,
                    )
                nc.vector.tensor_copy(
                    V_sb[:, NG * g : NG * (g + 1), 0:C],
                    vn.rearrange("p (j c) -> p j c", j=NG),
                )

            def scores_exp(qc, kp, exp_ref):
                """Quadrant-packed f32r scores pair + the exp for (qc, kp)."""
                qsl = slice(qc * QCHUNK, (qc + 1) * QCHUNK)
                sc = sc_pool.tile([P, 2 * QCHUNK], F32, tag="sc")
                for half, kblk in ((0, 2 * kp), (1, 2 * kp + 1)):
                    rows = slice(half * C, half * C + C)
                    ksl = slice(kblk * P, (kblk + 1) * P)
                    bank = slice(half * QCHUNK, (half + 1) * QCHUNK)
                    for lt, rt, st, sp in (
                        (NTh, MTh, True, False),
                        (NTh, MTl, False, False),
                        (NTl, MTh, False, True),
                    ):
                        nc.tensor.matmul(
                            sc[:, bank],
                            lhsT=lt[rows, ksl],
                            rhs=rt[rows, qsl],
                            start=st,
                            stop=sp,
                            tile_position=(half * C, 0),
                        )
                expS = work.tile([P, 2 * QCHUNK], BF16, tag="exps")
                nc.scalar.activation(
                    expS,
                    sc,
                    mybir.ActivationFunctionType.Exp,
                    bias=exp_bias,
                    scale=1.0,
                )
                exp_ref[kp] = expS

            def pv_step(pv_ps, kp, exp_ref):
                expS = exp_ref[kp]
                nc.tensor.matmul(
                    pv_ps,
                    lhsT=V_sb[:, 2 * kp, :],
                    rhs=expS[:, 0:QCHUNK],
                    start=(kp == 0),
                    stop=False,
                )
                nc.tensor.matmul(
                    pv_ps,
                    lhsT=V_sb[:, 2 * kp + 1, :],
                    rhs=expS[:, QCHUNK : 2 * QCHUNK],
                    start=False,
                    stop=(kp == NKP - 1),
                )

            def epilogue_block(qc, j, pv_sb):
                """Transpose one 128-q block of pv, normalize, add residual,
                DMA out (sync engine; idle during the inner loop)."""
                qb = qc * (QCHUNK // P) + j
                trx = misc.tile(
                    [P, QCHUNK], F32R, tag="mx", name=f"tr{qc}_{j}"
                )
                tr = trx[:, 0:VPAD]
                nc.tensor.transpose(
                    tr,
                    pv_sb[:, j * P : (j + 1) * P],
                    ident_r[0:VPAD, 0:VPAD],
                )
                trf = f(tr)
                rcp = ep.tile([P, 1], F32, tag="rcp")
                nc.vector.reciprocal(rcp, trf[:, C : C + 1])
                nc.vector.tensor_scalar_mul(o_sb[:, qb, :], trf[:, 0:C], rcp)
                nc.vector.tensor_add(
                    o_sb[:, qb, :], o_sb[:, qb, :], f(F_sb)[:, qb, :]
                )
                nc.sync.dma_start(
                    out=out_view[:, qb, :], in_=o_sb[:, qb, :]
                )

            # ---- fused schedule -------------------------------------------
            # qc0 absorbs the preamble: group g lands right before kps 2g,
            # 2g+1 which consume it. PV lags scores by one kp so the PE
            # never waits on the exp it just requested. Epilogue blocks of
            # qc are jammed between the early kps of qc+1.
            exp_ref = {}
            pv_tiles = {}
            ep_pending = []  # (qc, pv_sb) whose 4 blocks still need emitting

            def drain_epilogue(budget):
                while ep_pending and budget > 0:
                    eqc, pv_sb, jj = ep_pending[0]
                    epilogue_block(eqc, jj, pv_sb)
                    if jj == 3:
                        ep_pending.pop(0)
                    else:
                        ep_pending[0] = (eqc, pv_sb, jj + 1)
                    budget -= 1

            for qc in range(NQC):
                pv_tiles[qc] = pv_pool.tile(
                    [VPAD, QCHUNK], F32, tag="pv", name=f"pvacc{qc}"
                )
                for kp in range(NKP):
                    if qc == 0 and kp % 2 == 0 and kp // 2 < NQC:
                        preamble_group(kp // 2)
                    scores_exp(qc, kp, exp_ref)
                    if kp > 0:
                        pv_step(pv_tiles[qc], kp - 1, exp_ref)
                    elif qc > 0:
                        # first kp of a new qc: drain previous qc's epilogue
                        pv_sb = ep.tile([VPAD, QCHUNK], F32R, tag="pvsb")
                        nc.vector.tensor_copy(pv_sb, pv_tiles[qc - 1])
                        ep_pending.append((qc - 1, pv_sb, 0))
                    if qc > 0 and 1 <= kp <= 4:
                        drain_epilogue(1)
                pv_step(pv_tiles[qc], NKP - 1, exp_ref)

            # last qc epilogue
            pv_sb = ep.tile([VPAD, QCHUNK], F32R, tag="pvsb")
            nc.vector.tensor_copy(pv_sb, pv_tiles[NQC - 1])
            ep_pending.append((NQC - 1, pv_sb, 0))
            drain_epilogue(8)

    nc.finalize()
    return nc


_NC_CACHE = None


def _get_nc() -> bass.Bass:
    global _NC_CACHE
    if _NC_CACHE is None:
        _NC_CACHE = build_nc()
    return _NC_CACHE


def run_spmd(F, W_M, W_N, W_V, **kwargs):
    """Run the SPMD kernel; returns the BassKernelResults (for profiling)."""
    nc = _get_nc()
    in_maps = [
        {
            "F": np.ascontiguousarray(F[i], dtype=np.float32),
            "W_M": np.ascontiguousarray(W_M, dtype=np.float32),
            "W_N": np.ascontiguousarray(W_N, dtype=np.float32),
            "W_V": np.ascontiguousarray(W_V, dtype=np.float32),
        }
        for i in range(B)
    ]
    return run_bass_kernel_spmd(nc, in_maps, core_ids=list(range(B)), **kwargs)


def kernel(F, W_M, W_N, W_V):
    res = run_spmd(F, W_M, W_N, W_V)
    return np.stack([r["out"] for r in res.results]).astype(np.float32)


# revision 16
# speedup vs baseline: 1.0622x; 1.0622x over previous
"""Bass/Tile Trainium2 kernel for CrossPositionalAttention (v3: pipelined f32r).

Reference math (per batch element b):
    M = F @ W_M; N = F @ W_N; V = F @ W_V          # [T, C] each, T=2048, C=64
    S = softmax(M @ N^T, axis=-1)                  # [T, T]
    out = S @ V + F

Sharding: data-parallel over batch. B=8 == n_cores=8, so core i computes
batch element i end-to-end (no collectives); kernel() shards/gathers on host.

Design notes (v4):
  * Moving-operand bandwidth is ~2 B/lane/PE-cycle: bf16 rhs streams 1
    col/cycle at the full 2.4GHz clock, while f32r rhs is capped at the
    ~1.2GHz-equivalent rate (measured v3). So the inner loop uses bf16:
    scores = 3 bf16 passes on hi/lo splits (hi = bf16(x), lo = bf16(x-hi),
    ~17 bit combined: Nh.T@Mh + Nh.T@Ml + Nl.T@Mh), and PV consumes bf16
    expS with bf16 V. Preamble matmuls (projections, V natural, F^T
    transposes) stay f32r (f32r operands must be PRODUCED as f32r per the
    BIR verifier, hence bitcast dram APs / f32r-writing copies).
  * Permuted row order for fast DMA: F_sb[p, x, c] = F[16p + x, c] -- each
    partition loads 4KB contiguous. The permutation is applied consistently
    to M/N/V/scores/out and softmax is permutation-invariant over k, so
    writing out through the same view restores order.
  * ACT (scalar) does exp exclusively: [128,1024] psum->sbuf f32r per
    (qc,kp), ~1.04us saturated => ~33us floor; everything else is kept off
    ACT and the whole schedule aims to keep ACT saturated.
  * Software pipelining: PV(kp) is emitted after scores(kp+1) so the PE
    never waits on the exp it just requested; epilogue transposes of qc are
    jammed between the first kps of qc+1. The PE stays busy, which also
    holds the DVFS clock up (cold PE runs at 0.65GHz, warm at 2.4GHz).
  * Warmup: a few dummy ident transposes while the F DMA lands, so the
    PE clock ramps before the real preamble; a dummy exp on the scalar
    queue front-loads the ACT table load.
  * Preamble (F^T transposes -> M/N/V projections -> V natural transposes,
    by 4-block group) is interleaved with qc0's kp stream: group g is
    emitted before kps 2g, 2g+1 which consume it. psum->sbuf copies split
    between DVE and Pool so neither starves ACT.
  * DMA posting costs ~650ns per descriptor on an engine queue, so posts
    are spread: sync takes F chunks 0/2 + all output blocks, scalar (idle
    pre-exp) takes the weights and F chunks 1/3.
"""

import numpy as np

import concourse.bacc as bacc
import concourse.bass as bass
import concourse.tile as tile
from concourse import mybir
from concourse.bass_utils import run_bass_kernel_spmd
from concourse.masks import make_identity

B, T, C = 8, 2048, 64
P = 128
NBLK = T // P          # 16 blocks of 128 rows (permuted order)
QCHUNK = 512           # moving-operand free dim per matmul
NQC = T // QCHUNK      # 4 q-chunks
NG = 4                 # preamble block group size (4 blocks = 512 cols)
NKP = NBLK // 2        # 8 kp pairs per q-chunk
F32 = mybir.dt.float32
F32R = mybir.dt.float32r
BF16 = mybir.dt.bfloat16
EXP_BIAS = -40.0       # constant softmax shift (cancels in the normalization)
VPAD = 66              # V tile free dim: 64 V cols + ones col + pad (f32r: even)
NWARM = 2              # dummy transposes to ramp the PE clock


def build_nc() -> bass.Bass:
    nc = bacc.Bacc()
    F_h = nc.declare_dram_parameter("F", [T, C], F32, isOutput=False)
    Wm_h = nc.declare_dram_parameter("W_M", [C, C], F32, isOutput=False)
    Wn_h = nc.declare_dram_parameter("W_N", [C, C], F32, isOutput=False)
    Wv_h = nc.declare_dram_parameter("W_V", [C, C], F32, isOutput=False)
    out_h = nc.declare_dram_parameter("out", [T, C], F32, isOutput=True)

    # contiguous view: partition p holds rows 16p..16p+15 (4KB/partition)
    F_view = F_h[:, :].rearrange("(p x) c -> p x c", p=P)
    out_view = out_h[:, :].rearrange("(p x) c -> p x c", p=P)

    def r(ap):
        return ap.bitcast(F32R)

    def f(ap):
        return ap.bitcast(F32)

    with tile.TileContext(nc) as tc:
        with (
            tc.tile_pool(name="const", bufs=1) as const_pool,
            tc.tile_pool(name="persist", bufs=1) as persist,
            tc.tile_pool(name="sc_ps", bufs=2, space="PSUM") as sc_pool,
            tc.tile_pool(name="pv_ps", bufs=2, space="PSUM") as pv_pool,
            tc.tile_pool(name="misc_ps", bufs=2, space="PSUM") as misc,
            tc.tile_pool(name="work", bufs=3) as work,
            tc.tile_pool(name="ep", bufs=4) as ep,
        ):
            ident = const_pool.tile([P, P], F32, tag="ident")
            make_identity(nc, ident)
            ident_r = const_pool.tile([P, P], F32R, tag="identr")
            nc.vector.tensor_copy(ident_r, ident)

            exp_bias = const_pool.tile([P, 1], F32, tag="expbias")
            nc.vector.memset(exp_bias, EXP_BIAS)

            ones32 = const_pool.tile([P, 2 * NBLK], F32, tag="ones32")
            nc.vector.memset(ones32, 1.0)

            # DMA posts cost ~650ns each on an engine queue. Critical order:
            # sync posts F chunk 0 (first dependency) then Wv and the rest;
            # scalar posts the M/N weights (needed by the first projection),
            # the act-table dummy exp, then F chunks 1/3.
            F_sb = persist.tile([P, NBLK, C], F32R, tag="fsb")
            Wm2 = const_pool.tile([C, P], F32R, tag="wm2")
            Wn2 = const_pool.tile([C, P], F32R, tag="wn2")
            Wv_sb = const_pool.tile([C, C], F32R, tag="wv")

            def f_chunk(eng, g):
                eng.dma_start(
                    out=F_sb[:, NG * g : NG * (g + 1), :],
                    in_=r(F_view[:, NG * g : NG * (g + 1), :]),
                )

            f_chunk(nc.sync, 0)
            nc.sync.dma_start(out=Wv_sb[:, :], in_=r(Wv_h[:, :]))
            f_chunk(nc.sync, 2)
            nc.scalar.dma_start(out=Wm2[:, 0:C], in_=r(Wm_h[:, :]))
            nc.scalar.dma_start(out=Wm2[:, C:P], in_=r(Wm_h[:, :]))
            nc.scalar.dma_start(out=Wn2[:, 0:C], in_=r(Wn_h[:, :]))
            nc.scalar.dma_start(out=Wn2[:, C:P], in_=r(Wn_h[:, :]))
            # dummy exp front-loads the ACT function-table load
            dumex = const_pool.tile([P, 2], BF16, tag="dumex")
            nc.scalar.activation(
                dumex, ones32[:, 0:2], mybir.ActivationFunctionType.Exp
            )
            f_chunk(nc.scalar, 1)
            f_chunk(nc.scalar, 3)

            F_T = persist.tile([C, T], F32R, tag="ft")
            MTh = persist.tile([P, T], BF16, tag="mth")
            MTl = persist.tile([P, T], BF16, tag="mtl")
            NTh = persist.tile([P, T], BF16, tag="nth")
            NTl = persist.tile([P, T], BF16, tag="ntl")
            V_sb = persist.tile([P, NBLK, VPAD], BF16, tag="vsb")
            o_sb = persist.tile([P, NBLK, C], F32, tag="osb")

            # ones cols (64: softmax denominator via PV matmul, 65: pad for
            # f32r alignment), one strided copy from the fp32 ones tile
            nc.vector.tensor_copy(
                V_sb[:, :, C:VPAD],
                ones32.rearrange("p (n t) -> p n t", n=NBLK),
            )

            # dummy transposes ramp the PE clock while the F DMA lands.
            # ALL psum scratch (scores, preamble, epilogue transposes) shares
            # one 3-slot ring of 2-bank tiles ("ps" tag, 6 banks); pv gets 2.
            def ps_tile(nm):
                return misc.tile([P, 2 * QCHUNK], F32, tag="ps", name=nm)

            for w in range(NWARM):
                mx = ps_tile(f"warm{w}")
                nc.tensor.transpose(r(mx)[:, 0:P], ident_r, ident_r)

            def pre_A(g):
                """F^T transposes for blocks 4g..4g+3 + one copy."""
                gsl = slice(g * QCHUNK, (g + 1) * QCHUNK)
                mx = ps_tile(f"ftr{g}")
                ftr = r(mx)[0:C, 0:QCHUNK]
                for j in range(NG):
                    blk = NG * g + j
                    nc.tensor.transpose(
                        ftr[:, j * P : (j + 1) * P], F_sb[:, blk, :], ident_r
                    )
                nc.vector.tensor_copy(F_T[:, gsl], ftr)

            def pre_B(g):
                """M/N projections + bf16 hi/lo splits for chunk g."""
                gsl = slice(g * QCHUNK, (g + 1) * QCHUNK)
                for W2, hi, lo, pnm in (
                    (Wm2, MTh, MTl, "pm"),
                    (Wn2, NTh, NTl, "pn"),
                ):
                    mx = ps_tile(f"{pnm}{g}")
                    pp = mx[:, 0:QCHUNK]
                    nc.tensor.matmul(
                        pp, lhsT=W2, rhs=F_T[:, gsl], start=True, stop=True
                    )
                    nc.vector.tensor_copy(hi[:, gsl], pp)
                    nc.vector.tensor_tensor(
                        out=lo[:, gsl],
                        in0=pp,
                        in1=hi[:, gsl],
                        op=mybir.AluOpType.subtract,
                    )

            def pre_C(g):
                """V natural for blocks 4g..4g+3: lhsT = F^T block
                (stationary), rhs = W_V streamed; one bf16 copy out."""
                mx = ps_tile(f"vn{g}")
                vn = mx[:, 0 : NG * C]
                for j in range(NG):
                    blk = NG * g + j
                    nc.tensor.matmul(
                        vn[:, j * C : (j + 1) * C],
                        lhsT=F_T[:, blk * P : (blk + 1) * P],
                        rhs=Wv_sb,
                        start=True,
                        stop=True,
                    )
                nc.vector.tensor_copy(
                    V_sb[:, NG * g : NG * (g + 1), 0:C],
                    vn.rearrange("p (j c) -> p j c", j=NG),
                )

            def scores_exp(s, exp_ref):
                qc, kp = s // NKP, s % NKP
                qsl = slice(qc * QCHUNK, (qc + 1) * QCHUNK)
                sc = ps_tile(f"sc{s}")
                for half, kblk in ((0, 2 * kp), (1, 2 * kp + 1)):
                    rows = slice(half * C, half * C + C)
                    ksl = slice(kblk * P, (kblk + 1) * P)
                    bank = slice(half * QCHUNK, (half + 1) * QCHUNK)
                    for lt, rt, st, sp in (
                        (NTh, MTh, True, False),
                        (NTh, MTl, False, False),
                        (NTl, MTh, False, True),
                    ):
                        nc.tensor.matmul(
                            sc[:, bank],
                            lhsT=lt[rows, ksl],
                            rhs=rt[rows, qsl],
                            start=st,
                            stop=sp,
                            tile_position=(half * C, 0),
                        )
                expS = work.tile([P, 2 * QCHUNK], BF16, tag="exps")
                nc.scalar.activation(
                    expS,
                    sc,
                    mybir.ActivationFunctionType.Exp,
                    bias=exp_bias,
                    scale=1.0,
                )
                exp_ref[s] = expS

            def pv_step(pv_ps, s, exp_ref):
                kp = s % NKP
                expS = exp_ref.pop(s)
                nc.tensor.matmul(
                    pv_ps,
                    lhsT=V_sb[:, 2 * kp, :],
                    rhs=expS[:, 0:QCHUNK],
                    start=(kp == 0),
                    stop=False,
                )
                nc.tensor.matmul(
                    pv_ps,
                    lhsT=V_sb[:, 2 * kp + 1, :],
                    rhs=expS[:, QCHUNK : 2 * QCHUNK],
                    start=False,
                    stop=(kp == NKP - 1),
                )

            def epilogue_block(qc, j, pv_ps, pv_sb, post_eng):
                """Copy one 128-col slice of pv, transpose, normalize, add
                residual, DMA the block out."""
                qb = qc * (QCHUNK // P) + j
                csl = slice(j * P, (j + 1) * P)
                nc.vector.tensor_copy(pv_sb[:, csl], pv_ps[:, csl])
                trx = ps_tile(f"tr{qc}_{j}")
                tr = r(trx)[:, 0:VPAD]
                nc.tensor.transpose(
                    tr, pv_sb[:, csl], ident_r[0:VPAD, 0:VPAD]
                )
                trf = f(tr)
                rcp = ep.tile([P, 1], F32, tag="rcp")
                nc.vector.reciprocal(rcp, trf[:, C : C + 1])
                nc.vector.tensor_scalar_mul(o_sb[:, qb, :], trf[:, 0:C], rcp)
                nc.vector.tensor_add(
                    o_sb[:, qb, :], o_sb[:, qb, :], f(F_sb)[:, qb, :]
                )
                post_eng.dma_start(out=out_view[:, qb, :], in_=o_sb[:, qb, :])

            # ---- flat 32-step schedule --------------------------------------
            # step s: [qc0 preamble piece] scores+exp(s); pv(s-2); epilogue of
            # the previous qc drains one block per step early in each qc.
            NSTEP = NQC * NKP
            exp_ref = {}
            pv_tiles = {}
            pv_sbs = {}
            pre_A(0)
            pre_B(0)
            for s in range(NSTEP):
                qc, kp = s // NKP, s % NKP
                if qc == 0:
                    if kp in (0, 2, 4) and kp // 2 + 1 < NQC:
                        pre_A(kp // 2 + 1)
                    if kp in (1, 3, 5):
                        pre_B(kp // 2 + 1)
                        pre_C(kp // 2)
                    if kp == 6:
                        pre_C(3)
                scores_exp(s, exp_ref)
                t = s - 2
                if t >= 0:
                    tqc, tkp = t // NKP, t % NKP
                    if tkp == 0:
                        pv_tiles[tqc] = pv_pool.tile(
                            [VPAD, QCHUNK], F32, tag="pv", name=f"pvacc{tqc}"
                        )
                        if tqc > 0:
                            pv_sbs[tqc - 1] = ep.tile(
                                [VPAD, QCHUNK],
                                F32R,
                                tag="pvsb",
                                name=f"pvsb{tqc - 1}",
                            )
                    pv_step(pv_tiles[tqc], t, exp_ref)
                    if tqc > 0 and 1 <= tkp <= 4:
                        epilogue_block(
                            tqc - 1,
                            tkp - 1,
                            pv_tiles[tqc - 1],
                            pv_sbs[tqc - 1],
                            nc.sync,
                        )
            # drain: last two pv steps, then qc3 epilogue (scalar engine is
            # free after the last exp -- split the final DMA posts)
            for t in (NSTEP - 2, NSTEP - 1):
                pv_step(pv_tiles[NQC - 1], t, exp_ref)
            pv_sbs[NQC - 1] = ep.tile(
                [VPAD, QCHUNK], F32R, tag="pvsb", name=f"pvsb{NQC - 1}"
            )
            for j in range(QCHUNK // P):
                epilogue_block(
                    NQC - 1,
                    j,
                    pv_tiles[NQC - 1],
                    pv_sbs[NQC - 1],
                    nc.scalar if j % 2 == 0 else nc.sync,
                )

    nc.finalize()
    return nc


_NC_CACHE = None


def _get_nc() -> bass.Bass:
    global _NC_CACHE
    if _NC_CACHE is None:
        _NC_CACHE = build_nc()
    return _NC_CACHE


def run_spmd(F, W_M, W_N, W_V, **kwargs):
    """Run the SPMD kernel; returns the BassKernelResults (for profiling)."""
    nc = _get_nc()
    in_maps = [
        {
            "F": np.ascontiguousarray(F[i], dtype=np.float32),
            "W_M": np.ascontiguousarray(W_M, dtype=np.float32),
            "W_N": np.ascontiguousarray(W_N, dtype=np.float32),
            "W_V": np.ascontiguousarray(W_V, dtype=np.float32),
        }
        for i in range(B)
    ]
    return run_bass_kernel_spmd(nc, in_maps, core_ids=list(range(B)), **kwargs)


def kernel(F, W_M, W_N, W_V):
    res = run_spmd(F, W_M, W_N, W_V)
    return np.stack([r["out"] for r in res.results]).astype(np.float32)


# revision 17
# speedup vs baseline: 1.1078x; 1.0429x over previous
"""Bass/Tile Trainium2 kernel for CrossPositionalAttention (v3: pipelined f32r).

Reference math (per batch element b):
    M = F @ W_M; N = F @ W_N; V = F @ W_V          # [T, C] each, T=2048, C=64
    S = softmax(M @ N^T, axis=-1)                  # [T, T]
    out = S @ V + F

Sharding: data-parallel over batch. B=8 == n_cores=8, so core i computes
batch element i end-to-end (no collectives); kernel() shards/gathers on host.

Design notes (v4):
  * Moving-operand bandwidth is ~2 B/lane/PE-cycle: bf16 rhs streams 1
    col/cycle at the full 2.4GHz clock, while f32r rhs is capped at the
    ~1.2GHz-equivalent rate (measured v3). So the inner loop uses bf16:
    scores = 3 bf16 passes on hi/lo splits (hi = bf16(x), lo = bf16(x-hi),
    ~17 bit combined: Nh.T@Mh + Nh.T@Ml + Nl.T@Mh), and PV consumes bf16
    expS with bf16 V. Preamble matmuls (projections, V natural, F^T
    transposes) stay f32r (f32r operands must be PRODUCED as f32r per the
    BIR verifier, hence bitcast dram APs / f32r-writing copies).
  * Permuted row order for fast DMA: F_sb[p, x, c] = F[16p + x, c] -- each
    partition loads 4KB contiguous. The permutation is applied consistently
    to M/N/V/scores/out and softmax is permutation-invariant over k, so
    writing out through the same view restores order.
  * ACT (scalar) does exp exclusively: [128,1024] psum->sbuf f32r per
    (qc,kp), ~1.04us saturated => ~33us floor; everything else is kept off
    ACT and the whole schedule aims to keep ACT saturated.
  * Software pipelining: PV(kp) is emitted after scores(kp+1) so the PE
    never waits on the exp it just requested; epilogue transposes of qc are
    jammed between the first kps of qc+1. The PE stays busy, which also
    holds the DVFS clock up (cold PE runs at 0.65GHz, warm at 2.4GHz).
  * Warmup: a few dummy ident transposes while the F DMA lands, so the
    PE clock ramps before the real preamble; a dummy exp on the scalar
    queue front-loads the ACT table load.
  * Preamble (F^T transposes -> M/N/V projections -> V natural transposes,
    by 4-block group) is interleaved with qc0's kp stream: group g is
    emitted before kps 2g, 2g+1 which consume it. psum->sbuf copies split
    between DVE and Pool so neither starves ACT.
  * DMA posting costs ~650ns per descriptor on an engine queue, so posts
    are spread: sync takes F chunks 0/2 + all output blocks, scalar (idle
    pre-exp) takes the weights and F chunks 1/3.
"""

import numpy as np

import concourse.bacc as bacc
import concourse.bass as bass
import concourse.tile as tile
from concourse import mybir
from concourse.bass_utils import run_bass_kernel_spmd
from concourse.masks import make_identity

B, T, C = 8, 2048, 64
P = 128
NBLK = T // P          # 16 blocks of 128 rows (permuted order)
QCHUNK = 512           # moving-operand free dim per matmul
NQC = T // QCHUNK      # 4 q-chunks
NG = 4                 # preamble block group size (4 blocks = 512 cols)
NKP = NBLK // 2        # 8 kp pairs per q-chunk
F32 = mybir.dt.float32
F32R = mybir.dt.float32r
BF16 = mybir.dt.bfloat16
EXP_BIAS = -40.0       # constant softmax shift (cancels in the normalization)
VPAD = 66              # V tile free dim: 64 V cols + ones col + pad (f32r: even)
NWARM = 14             # junk matmuls to ramp the PE clock


def build_nc() -> bass.Bass:
    nc = bacc.Bacc()
    F_h = nc.declare_dram_parameter("F", [T, C], F32, isOutput=False)
    Wm_h = nc.declare_dram_parameter("W_M", [C, C], F32, isOutput=False)
    Wn_h = nc.declare_dram_parameter("W_N", [C, C], F32, isOutput=False)
    Wv_h = nc.declare_dram_parameter("W_V", [C, C], F32, isOutput=False)
    out_h = nc.declare_dram_parameter("out", [T, C], F32, isOutput=True)

    # contiguous view: partition p holds rows 16p..16p+15 (4KB/partition)
    F_view = F_h[:, :].rearrange("(p x) c -> p x c", p=P)
    out_view = out_h[:, :].rearrange("(p x) c -> p x c", p=P)

    def r(ap):
        return ap.bitcast(F32R)

    def f(ap):
        return ap.bitcast(F32)

    with tile.TileContext(nc) as tc:
        with (
            tc.tile_pool(name="const", bufs=1) as const_pool,
            tc.tile_pool(name="persist", bufs=1) as persist,
            tc.tile_pool(name="sc_ps", bufs=2, space="PSUM") as sc_pool,
            tc.tile_pool(name="pv_ps", bufs=2, space="PSUM") as pv_pool,
            tc.tile_pool(name="aux_ps", bufs=2, space="PSUM") as aux,
            tc.tile_pool(name="work", bufs=3) as work,
            tc.tile_pool(name="ep", bufs=4) as ep,
        ):
            ident = const_pool.tile([P, P], F32, tag="ident")
            make_identity(nc, ident)
            ident_r = const_pool.tile([P, P], F32R, tag="identr")
            nc.vector.tensor_copy(ident_r, ident)

            exp_bias = const_pool.tile([P, 1], F32, tag="expbias")
            nc.vector.memset(exp_bias, EXP_BIAS)

            ones32 = const_pool.tile([P, 2 * NBLK], F32, tag="ones32")
            nc.vector.memset(ones32, 1.0)

            # DMA posts cost ~650ns each on an engine queue. Critical order:
            # sync posts F chunk 0 (first dependency) then Wv and the rest;
            # scalar posts the M/N weights (needed by the first projection),
            # the act-table dummy exp, then F chunks 1/3.
            F_sb = persist.tile([P, NBLK, C], F32R, tag="fsb")
            Wm2 = const_pool.tile([C, P], F32R, tag="wm2")
            Wn2 = const_pool.tile([C, P], F32R, tag="wn2")
            Wv_sb = const_pool.tile([C, C], F32R, tag="wv")

            def f_chunk(eng, g):
                eng.dma_start(
                    out=F_sb[:, NG * g : NG * (g + 1), :],
                    in_=r(F_view[:, NG * g : NG * (g + 1), :]),
                )

            f_chunk(nc.sync, 0)
            nc.sync.dma_start(out=Wv_sb[:, :], in_=r(Wv_h[:, :]))
            f_chunk(nc.sync, 2)
            nc.scalar.dma_start(out=Wm2[:, 0:C], in_=r(Wm_h[:, :]))
            nc.scalar.dma_start(out=Wm2[:, C:P], in_=r(Wm_h[:, :]))
            nc.scalar.dma_start(out=Wn2[:, 0:C], in_=r(Wn_h[:, :]))
            nc.scalar.dma_start(out=Wn2[:, C:P], in_=r(Wn_h[:, :]))
            # dummy exp front-loads the ACT function-table load
            dumex = const_pool.tile([P, 2], BF16, tag="dumex")
            nc.scalar.activation(
                dumex, ones32[:, 0:2], mybir.ActivationFunctionType.Exp
            )
            f_chunk(nc.scalar, 1)
            f_chunk(nc.scalar, 3)

            F_T = persist.tile([C, T], F32R, tag="ft")
            MTh = persist.tile([P, T], BF16, tag="mth")
            MTl = persist.tile([P, T], BF16, tag="mtl")
            NTh = persist.tile([P, T], BF16, tag="nth")
            NTl = persist.tile([P, T], BF16, tag="ntl")
            V_sb = persist.tile([P, NBLK, VPAD], BF16, tag="vsb")
            o_sb = persist.tile([P, NBLK, C], F32, tag="osb")

            # ones cols (64: softmax denominator via PV matmul, 65: pad for
            # f32r alignment), one strided copy from the fp32 ones tile
            nc.vector.tensor_copy(
                V_sb[:, :, C:VPAD],
                ones32.rearrange("p (n t) -> p n t", n=NBLK),
            )

            # PSUM: scores ring 2 x [128,1024] (4 banks; 2 slots suffice --
            # ACT is the bottleneck), pv 2 x [66,512] (2 banks), aux ring
            # 2 x 1-bank slots for warmup/preamble pieces/epilogue transposes.
            def ps_tile(nm):
                return sc_pool.tile([P, 2 * QCHUNK], F32, tag="sc", name=nm)

            def aux_tile(nm):
                return aux.tile([P, QCHUNK], F32, tag="aux", name=nm)

            # junk matmuls ramp the PE clock while the F DMA lands; they
            # depend only on the ones tile (ready ~t=5.7us), not on ident
            for w in range(NWARM):
                jk = aux_tile(f"warm{w}")
                nc.tensor.matmul(
                    jk[0:32, 0:32],
                    lhsT=ones32[0:C, 0:32],
                    rhs=ones32[0:C, 0:32],
                    start=True,
                    stop=True,
                )

            def pre_A(g):
                """F^T transposes for blocks 4g..4g+3 + one copy."""
                gsl = slice(g * QCHUNK, (g + 1) * QCHUNK)
                mx = aux_tile(f"ftr{g}")
                ftr = r(mx)[0:C, 0:QCHUNK]
                for j in range(NG):
                    blk = NG * g + j
                    nc.tensor.transpose(
                        ftr[:, j * P : (j + 1) * P], F_sb[:, blk, :], ident_r
                    )
                nc.vector.tensor_copy(F_T[:, gsl], ftr)

            def pre_B(g):
                """M/N projections + bf16 hi/lo splits for chunk g."""
                gsl = slice(g * QCHUNK, (g + 1) * QCHUNK)
                for W2, hi, lo, pnm in (
                    (Wm2, MTh, MTl, "pm"),
                    (Wn2, NTh, NTl, "pn"),
                ):
                    mx = aux_tile(f"{pnm}{g}")
                    pp = mx[:, 0:QCHUNK]
                    nc.tensor.matmul(
                        pp, lhsT=W2, rhs=F_T[:, gsl], start=True, stop=True
                    )
                    nc.vector.tensor_copy(hi[:, gsl], pp)
                    nc.vector.tensor_tensor(
                        out=lo[:, gsl],
                        in0=pp,
                        in1=hi[:, gsl],
                        op=mybir.AluOpType.subtract,
                    )

            def pre_C(g):
                """V natural for blocks 4g..4g+3: lhsT = F^T block
                (stationary), rhs = W_V streamed; one bf16 copy out."""
                mx = aux_tile(f"vn{g}")
                vn = mx[:, 0 : NG * C]
                for j in range(NG):
                    blk = NG * g + j
                    nc.tensor.matmul(
                        vn[:, j * C : (j + 1) * C],
                        lhsT=F_T[:, blk * P : (blk + 1) * P],
                        rhs=Wv_sb,
                        start=True,
                        stop=True,
                    )
                nc.vector.tensor_copy(
                    V_sb[:, NG * g : NG * (g + 1), 0:C],
                    vn.rearrange("p (j c) -> p j c", j=NG),
                )

            def scores_exp(s, exp_ref):
                qc, kp = s // NKP, s % NKP
                qsl = slice(qc * QCHUNK, (qc + 1) * QCHUNK)
                sc = ps_tile(f"sc{s}")
                for half, kblk in ((0, 2 * kp), (1, 2 * kp + 1)):
                    rows = slice(half * C, half * C + C)
                    ksl = slice(kblk * P, (kblk + 1) * P)
                    bank = slice(half * QCHUNK, (half + 1) * QCHUNK)
                    for lt, rt, st, sp in (
                        (NTh, MTh, True, False),
                        (NTh, MTl, False, False),
                        (NTl, MTh, False, True),
                    ):
                        nc.tensor.matmul(
                            sc[:, bank],
                            lhsT=lt[rows, ksl],
                            rhs=rt[rows, qsl],
                            start=st,
                            stop=sp,
                            tile_position=(half * C, 0),
                        )
                expS = work.tile([P, 2 * QCHUNK], BF16, tag="exps")
                nc.scalar.activation(
                    expS,
                    sc,
                    mybir.ActivationFunctionType.Exp,
                    bias=exp_bias,
                    scale=1.0,
                )
                exp_ref[s] = expS

            def pv_step(pv_ps, s, exp_ref):
                kp = s % NKP
                expS = exp_ref.pop(s)
                nc.tensor.matmul(
                    pv_ps,
                    lhsT=V_sb[:, 2 * kp, :],
                    rhs=expS[:, 0:QCHUNK],
                    start=(kp == 0),
                    stop=False,
                )
                nc.tensor.matmul(
                    pv_ps,
                    lhsT=V_sb[:, 2 * kp + 1, :],
                    rhs=expS[:, QCHUNK : 2 * QCHUNK],
                    start=False,
                    stop=(kp == NKP - 1),
                )

            def epilogue_block(qc, j, pv_ps, pv_sb, post_eng):
                """Copy one 128-col slice of pv, transpose, normalize, add
                residual, DMA the block out."""
                qb = qc * (QCHUNK // P) + j
                csl = slice(j * P, (j + 1) * P)
                nc.vector.tensor_copy(pv_sb[:, csl], pv_ps[:, csl])
                trx = aux_tile(f"tr{qc}_{j}")
                tr = r(trx)[:, 0:VPAD]
                nc.tensor.transpose(
                    tr, pv_sb[:, csl], ident_r[0:VPAD, 0:VPAD]
                )
                trf = f(tr)
                rcp = ep.tile([P, 1], F32, tag="rcp")
                nc.vector.reciprocal(rcp, trf[:, C : C + 1])
                nc.vector.tensor_scalar_mul(o_sb[:, qb, :], trf[:, 0:C], rcp)
                nc.vector.tensor_add(
                    o_sb[:, qb, :], o_sb[:, qb, :], f(F_sb)[:, qb, :]
                )
                post_eng.dma_start(out=out_view[:, qb, :], in_=o_sb[:, qb, :])

            # ---- flat 32-step schedule --------------------------------------
            # step s: [qc0 preamble piece] scores+exp(s); pv(s-2); epilogue of
            # the previous qc drains one block per step early in each qc.
            NSTEP = NQC * NKP
            exp_ref = {}
            pv_tiles = {}
            pv_sbs = {}
            pre_A(0)
            pre_B(0)
            for s in range(NSTEP):
                qc, kp = s // NKP, s % NKP
                if qc == 0:
                    if kp in (0, 2, 4) and kp // 2 + 1 < NQC:
                        pre_A(kp // 2 + 1)
                    if kp in (1, 3, 5):
                        pre_B(kp // 2 + 1)
                        pre_C(kp // 2)
                    if kp == 6:
                        pre_C(3)
                scores_exp(s, exp_ref)
                t = s - 2
                if t >= 0:
                    tqc, tkp = t // NKP, t % NKP
                    if tkp == 0:
                        pv_tiles[tqc] = pv_pool.tile(
                            [VPAD, QCHUNK], F32, tag="pv", name=f"pvacc{tqc}"
                        )
                        if tqc > 0:
                            pv_sbs[tqc - 1] = ep.tile(
                                [VPAD, QCHUNK],
                                F32R,
                                tag="pvsb",
                                name=f"pvsb{tqc - 1}",
                            )
                    pv_step(pv_tiles[tqc], t, exp_ref)
                    if tqc > 0 and 1 <= tkp <= 4:
                        epilogue_block(
                            tqc - 1,
                            tkp - 1,
                            pv_tiles[tqc - 1],
                            pv_sbs[tqc - 1],
                            nc.sync,
                        )
            # drain: last two pv steps, then qc3 epilogue (scalar engine is
            # free after the last exp -- split the final DMA posts)
            for t in (NSTEP - 2, NSTEP - 1):
                pv_step(pv_tiles[NQC - 1], t, exp_ref)
            pv_sbs[NQC - 1] = ep.tile(
                [VPAD, QCHUNK], F32R, tag="pvsb", name=f"pvsb{NQC - 1}"
            )
            for j in range(QCHUNK // P):
                epilogue_block(
                    NQC - 1,
                    j,
                    pv_tiles[NQC - 1],
                    pv_sbs[NQC - 1],
                    nc.scalar if j % 2 == 0 else nc.sync,
                )

    nc.finalize()
    return nc


_NC_CACHE = None


def _get_nc() -> bass.Bass:
    global _NC_CACHE
    if _NC_CACHE is None:
        _NC_CACHE = build_nc()
    return _NC_CACHE


def run_spmd(F, W_M, W_N, W_V, **kwargs):
    """Run the SPMD kernel; returns the BassKernelResults (for profiling)."""
    nc = _get_nc()
    in_maps = [
        {
            "F": np.ascontiguousarray(F[i], dtype=np.float32),
            "W_M": np.ascontiguousarray(W_M, dtype=np.float32),
            "W_N": np.ascontiguousarray(W_N, dtype=np.float32),
            "W_V": np.ascontiguousarray(W_V, dtype=np.float32),
        }
        for i in range(B)
    ]
    return run_bass_kernel_spmd(nc, in_maps, core_ids=list(range(B)), **kwargs)


def kernel(F, W_M, W_N, W_V):
    res = run_spmd(F, W_M, W_N, W_V)
    return np.stack([r["out"] for r in res.results]).astype(np.float32)
